# revision 1
# baseline (speedup 1.0000x reference)
"""CaptioningRNN (LSTM + spatial attention + vocab loss) on 8 Trainium2 cores.

Strategy:
 - The three big parallel matmul groups (CNN-feature projection, x@Wx
   precompute, vocab logits + logsumexp) are sharded 8 ways.
 - The sequential LSTM is sharded over the 4H gate dimension: core c owns a
   128-wide slice of each gate (aligned with hidden slice c). Each step ends
   with a fused AllGather carrying (a) the core's transposed h-slice (feeds
   the next step's matmuls on every core) and (b) partial attention scores
   for the core's hidden slice (summed on every core after the gather).
 - Attention:  scores_partial = ones^T @ (A^T_slice * h^T_slice)  on PE,
   attn @ Wattn is eliminated via B[n,p,:] = A[n,:,p] @ Wattn precompute and
   a per-position scalar_tensor_tensor FMA chain on DVE.
 - Loss: logsumexp partials per vocab shard (no max subtraction - logits are
   O(3)), label logits via host-gathered W_vocab columns, one final AllGather,
   then a replicated tiny reduction. b_vocab[y] mask term added on host.
"""
import sys, os, time

sys.path.insert(0, "/opt/trn_rl_repo")

import numpy as np
import ml_dtypes

import concourse.bass as bass
import concourse.bacc as bacc
import concourse.mybir as mybir
import concourse.tile as tile
import concourse.bass_isa as bass_isa
from concourse.bass_utils import run_bass_kernel_spmd


BF16 = ml_dtypes.bfloat16
F32 = mybir.dt.float32
BF = mybir.dt.bfloat16
U8 = mybir.dt.uint8

NCORES = 8
N = 128          # batch
TCAP = 31
CIN = 1280
WD = 512         # embed dim
H = 1024         # hidden
V = 10000        # vocab
P = 16           # spatial positions
HS = H // NCORES      # 128 hidden slice
SC = 4 * HS           # 512 a-columns per core
VS = V // NCORES      # 1250 vocab slice
AluOp = mybir.AluOpType
Act = mybir.ActivationFunctionType

HT_BYTES = 128 * 128 * 2           # h^T slice, bf16
SC_BYTES = 2048 * 4                # partial scores, fp32
PAY = HT_BYTES + SC_BYTES          # per-step AG payload bytes


def build(T):
    STAGE = int(os.environ.get("KSTAGE", "4"))
    DBG = os.environ.get("KDEBUG", "0") == "1"
    nc = bacc.Bacc("TRN2", target_bir_lowering=False, debug=False,
                   num_devices=NCORES)
    NT = T * N

    def din(name, shape, dt):
        return nc.dram_tensor(name, shape, dt, kind="ExternalInput").ap()

    imgsT = din("imgsT", [CIN, N * P], BF)
    wprojS = din("wprojS", [CIN, HS], BF)
    bprojS = din("bprojS", [HS, 1], F32)
    xembT = din("xembT", [WD, NT], BF)
    WxS = din("WxS", [WD, SC], BF)
    bSb = din("bSb", [N, SC], F32)
    WhS = din("WhS", [H, SC], BF)
    WattnS = din("WattnS", [H, SC], BF)
    wvoc = din("wvoc", [H, VS], BF)
    bvoc = din("bvoc", [1, VS], BF)
    wyT = din("wyT", [T, 128, H], BF)
    mask_f = din("mask_f", [N, T], F32)
    ident = din("ident", [128, 128], BF)
    ones32 = din("ones32", [128, 32], BF)  # value 1/32 (scores scale)
    ones1 = din("ones1", [128, 1], BF)
    ones1f = din("ones1f", [128, 1], F32)
    onesrow = din("onesrow", [1, 128], BF)
    identf = din("identf", [128, 128], F32)

    loss_out = nc.dram_tensor("loss", [1, 1], F32, kind="ExternalOutput").ap()
    if DBG:
        dbg_h0t = nc.dram_tensor("dbg_h0t", [128, 128], F32, kind="ExternalOutput").ap()
        dbg_sc = nc.dram_tensor("dbg_sc", [128, P], F32, kind="ExternalOutput").ap()
        dbg_w = nc.dram_tensor("dbg_w", [128, P], F32, kind="ExternalOutput").ap()
        dbg_a = nc.dram_tensor("dbg_a", [128, SC], F32, kind="ExternalOutput").ap()
        dbg_h1 = nc.dram_tensor("dbg_h1", [128, 128], F32, kind="ExternalOutput").ap()
        dbg_z = nc.dram_tensor("dbg_z", [128, T], F32, kind="ExternalOutput").ap()


    VCH = [(0, 512), (512, 512), (1024, VS - 1024)]  # vocab chunks
    ZB = N * T * 4
    rg = [list(range(NCORES))]

    with tile.TileContext(nc) as tc:
        with (
            tc.tile_pool(name="dram", bufs=1, space="DRAM") as dram,
            tc.tile_pool(name="persist", bufs=1) as pp,
            tc.tile_pool(name="work", bufs=3) as wp,
            tc.tile_pool(name="psB", bufs=1, space="PSUM") as psB,
        ):
            # ---------- persistent SBUF ----------
            atm = pp.tile([128, N, P], BF, name="atm")        # A^T my-slice (n,p)
            at_all = pp.tile([128, 8, N, P], BF, name="at_all")
            b_sb = pp.tile([128, P, SC], BF, name="b_sb")
            whs_sb = pp.tile([128, 8, SC], BF, name="whs_sb")
            wvoc_sb = pp.tile([128, 8, VS], BF, name="wvoc_sb")
            bvoc_sb = pp.tile([1, VS], BF, name="bvoc_sb")
            xwx_dram = dram.tile([T, 128, SC], BF, name="xwx_dram")
            zc_sb = pp.tile([128, T, 3], F32, name="zc_sb")

            c_sb = pp.tile([128, 128], F32, name="c_sb")
            ident_sb = pp.tile([128, 128], BF, name="ident_sb")
            ones32_sb = pp.tile([128, 32], BF, name="ones32_sb")
            ones1_sb = pp.tile([128, 1], BF, name="ones1_sb")
            ones1f_sb = pp.tile([128, 1], F32, name="ones1f_sb")
            onesrow_sb = pp.tile([1, 128], BF, name="onesrow_sb")
            identf_sb = pp.tile([128, 128], F32, name="identf_sb")
            bproj_sb = pp.tile([HS, 1], F32, name="bproj_sb")
            mask_sb = pp.tile([N, T], F32, name="mask_sb")

            for dst, src in [(ident_sb, ident), (ones32_sb, ones32),
                             (ones1_sb, ones1), (ones1f_sb, ones1f),
                             (onesrow_sb, onesrow), (identf_sb, identf),
                             (bproj_sb, bprojS),
                             (mask_sb, mask_f), (bvoc_sb, bvoc)]:
                nc.sync.dma_start(dst[:], src[:])
            for kt in range(8):
                nc.sync.dma_start(whs_sb[:, kt, :], WhS[bass.ts(kt, 128), :])
                nc.sync.dma_start(wvoc_sb[:, kt, :], wvoc[bass.ts(kt, 128), :])

            # ---------- P1: projection  A^T[my hslice, (n,p)] ----------
            # 4-bank PSUM slot shared (tag ps_big) with per-step score sums,
            # label-logit sums and the final loss matmul.
            ps_a = psB.tile([128, N * P], F32, name="ps_a", tag="ps_big")
            wproj_sb = pp.tile([128, 10, 128], BF, name="wproj_sb")
            for kt in range(10):
                nc.sync.dma_start(wproj_sb[:, kt, :], wprojS[bass.ts(kt, 128), :])
            for kt in range(10):
                imgs_kt = wp.tile([128, N * P], BF, name=f"imgs{kt}",
                                  tag="imgs_kt", bufs=2)
                nc.sync.dma_start(imgs_kt[:], imgsT[bass.ts(kt, 128), :])
                for ch in range(4):
                    nc.tensor.matmul(ps_a[:, bass.ts(ch, 512)],
                                     wproj_sb[:, kt, :],
                                     imgs_kt[:, bass.ts(ch, 512)],
                                     start=(kt == 0), stop=(kt == 9))
            # + b_proj (per-partition), cast bf16
            nc.scalar.activation(atm[:].rearrange("p n q -> p (n q)"),
                                 ps_a[:], Act.Identity, bias=bproj_sb[:])
            # h0^T my slice = mean over positions
            h0t_f = pp.tile([128, 128], F32, name="h0t_f")
            nc.vector.tensor_reduce(h0t_f[:], atm[:], mybir.AxisListType.X,
                                    AluOp.add)
            nc.scalar.mul(h0t_f[:], h0t_f[:], 1.0 / P)
            h0t_b = pp.tile([128, 128], BF, name="h0t_b")
            nc.scalar.copy(h0t_b[:], h0t_f[:])
            if DBG:
                nc.sync.dma_start(dbg_h0t[:], h0t_f[:])
            # c0 = h0 (batch-major my hidden slice)
            ps_tr0 = psB.tile([128, 128], BF, name="ps_tr", tag="ps_tr",
                              bufs=1)
            nc.tensor.transpose(ps_tr0[:], h0t_b[:], ident_sb[:])
            nc.scalar.copy(c_sb[:], ps_tr0[:])

            # scores partial for h0 + payload + AG#0.
            # ones-trick: lhsT = (1/32)*ones [128,32] -> each 32-row group of
            # the PSUM gets the column sums; rows {0,32,64,96} hold the 4
            # chunks of the 2048-wide partial-score vector.
            def scores_and_ag(step, hT_bf):
                e_sb = wp.tile([128, N, P], BF, name=f"e_{step}", tag="e_sb", bufs=2)
                nc.vector.tensor_mul(
                    e_sb[:], atm[:],
                    hT_bf[:].unsqueeze(2).broadcast_to([128, 128, P]))
                ps_sc = psB.tile([128, 512], F32, name=f"ps_sc{step}",
                                 tag="ps_sc32", bufs=1)
                ev = e_sb[:].rearrange("p n q -> p (n q)")
                for ch in range(4):
                    nc.tensor.matmul(ps_sc[32 * ch:32 * (ch + 1), :],
                                     ones32_sb[:],
                                     ev[:, bass.ts(ch, 512)],
                                     start=True, stop=True,
                                     tile_position=(0, 32 * ch))
                sc_out = wp.tile([128, 512], F32, name=f"sco{step}",
                                 tag="sc_out", bufs=2)
                nc.vector.tensor_copy(sc_out[:], ps_sc[:])
                pay = dram.tile([PAY], U8, name=f"pay{step}")
                nc.sync.dma_start(
                    pay[0:HT_BYTES].rearrange("(p b) -> p b", p=128),
                    hT_bf[:].bitcast(mybir.dt.uint8))
                nc.sync.dma_start(
                    pay[HT_BYTES:PAY].bitcast(F32)
                    .rearrange("(c f) -> c f", c=4),
                    sc_out[0:128:32, :])
                gat = dram.tile([NCORES, PAY], U8, name=f"gat{step}")
                nc.gpsimd.collective_compute(
                    "AllGather", AluOp.bypass, replica_groups=rg,
                    ins=[pay.opt()], outs=[gat.opt()])
                return gat

            gat = scores_and_ag(0, h0t_b)

            # ---------- at_all: gather A^T from all cores ----------
            pay_a = dram.tile([128, N * P], BF, name="pay_a")
            nc.sync.dma_start(pay_a[:], atm[:].rearrange("p n q -> p (n q)"))
            gat_a = dram.tile([NCORES * 128, N * P], BF, name="gat_a")
            nc.gpsimd.collective_compute(
                "AllGather", AluOp.bypass, replica_groups=rg,
                ins=[pay_a.opt()], outs=[gat_a.opt()])
            gav = gat_a[:].rearrange("(c p) f -> c p f", c=NCORES)
            for kt in range(8):
                nc.sync.dma_start(
                    at_all[:, kt, :, :].rearrange("p n q -> p (n q)"), gav[kt])

            if STAGE >= 2:
                # ---------- B precompute: B[n,p,:] = A[n,:,p] @ WattnS ----------
                watt_sb = pp.tile([128, 8, SC], BF, name="watt_sb")
                for kt in range(8):
                    nc.sync.dma_start(watt_sb[:, kt, :], WattnS[bass.ts(kt, 128), :])
                for p in range(P):
                    ps_b = psB.tile([128, SC], F32, name=f"ps_b{p}", tag="ps_mm",
                                    bufs=2)
                    for kt in range(8):
                        nc.tensor.matmul(ps_b[:], at_all[:, kt, :, p],
                                         watt_sb[:, kt, :],
                                         start=(kt == 0), stop=(kt == 7))
                    nc.scalar.copy(b_sb[:, p, :], ps_b[:])

                # ---------- P2: xwxb[t] = x_t @ WxS + b ----------
                wxs_sb = pp.tile([128, 4, SC], BF, name="wxs_sb")
                bsb_sb = pp.tile([128, SC], F32, name="bsb_sb")
                nc.sync.dma_start(bsb_sb[:], bSb[:])
                for kt in range(4):
                    nc.sync.dma_start(wxs_sb[:, kt, :], WxS[bass.ts(kt, 128), :])
                for t in range(T):
                    xt_sb = wp.tile([128, 4, 128], BF, name=f"xt{t}", tag="xt_sb", bufs=2)
                    for kt in range(4):
                        nc.sync.dma_start(xt_sb[:, kt, :],
                                          xembT[bass.ts(kt, 128), bass.ts(t, 128)])
                    ps_x = psB.tile([128, SC], F32, name=f"ps_x{t}", tag="ps_mm",
                                    bufs=2)
                    for kt in range(4):
                        nc.tensor.matmul(ps_x[:], xt_sb[:, kt, :], wxs_sb[:, kt, :],
                                         start=(kt == 0), stop=(kt == 3))
                    xwx_tmp = wp.tile([128, SC], BF, name=f"xwxt{t}",
                                      tag="xwx_tmp", bufs=2)
                    nc.vector.tensor_add(xwx_tmp[:], ps_x[:], bsb_sb[:])
                    nc.sync.dma_start(xwx_dram[t], xwx_tmp[:])

            # ---------- recurrence ----------
            pay2 = dram.tile([N * T * 4 + T * 128 * 4], U8, name="pay2")

            def vocab_row(trow, hT_sb):
                # logsumexp partials + label logits for hs row `trow` (h_{trow+1})
                for ci, (off, ln) in enumerate(VCH):
                    ps_v = psB.tile([128, 512], F32, name=f"ps_v{trow}_{ci}",
                                    tag="ps_mm", bufs=2)
                    for kt in range(8):
                        nc.tensor.matmul(ps_v[:, :ln], hT_sb[:, kt, :],
                                         wvoc_sb[:, kt, off:off + ln],
                                         start=(kt == 0), stop=False)
                    nc.tensor.matmul(ps_v[:, :ln], onesrow_sb[:],
                                     bvoc_sb[:, off:off + ln],
                                     start=False, stop=True)
                    ex_scr = wp.tile([128, 512], BF, name=f"ex{trow}_{ci}",
                                     tag="ex_scr", bufs=2)
                    nc.scalar.activation(ex_scr[:, :ln], ps_v[:, :ln], Act.Exp,
                                         accum_out=zc_sb[:, trow, ci:ci + 1])
                # label logit
                wyt_sb = wp.tile([128, H], BF, name=f"wyt{trow}", tag="wyt_sb", bufs=2)
                nc.sync.dma_start(wyt_sb[:], wyT[trow])
                ey_sb = wp.tile([128, H], BF, name=f"ey{trow}", tag="ey_sb", bufs=2)
                nc.vector.tensor_mul(
                    ey_sb[:].rearrange("p (n k) -> p n k", k=8),
                    hT_sb[:].transpose([0, 2, 1]),
                    wyt_sb[:].rearrange("p (n k) -> p n k", k=8))
                ps_ll = psB.tile([1, 1024], F32, name=f"ps_ll{trow}",
                                 tag="ps_big", bufs=1)
                for ch in range(2):
                    nc.tensor.matmul(ps_ll[:, bass.ts(ch, 512)], ones1_sb[:],
                                     ey_sb[:, bass.ts(ch, 512)],
                                     start=True, stop=True)
                ll_t = wp.tile([1, 128], F32, name=f"ll_t{trow}",
                               tag="ll_t", bufs=2)
                nc.vector.tensor_reduce(
                    ll_t[:],
                    ps_ll[:].rearrange("o (n k) -> o n k", k=8),
                    mybir.AxisListType.X, AluOp.add)
                nc.sync.dma_start(
                    pay2[ZB:].bitcast(F32)
                    .rearrange("(n t) -> t n", n=128)[trow:trow + 1, :],
                    ll_t[:])

            if STAGE == 3:
                nc.vector.memset(zc_sb[:], 0.0)
            if STAGE >= 3:
                for t in range(T):
                    xwxt_sb = wp.tile([128, SC], BF, name=f"xwxl{t}",
                                      tag="xwxt_sb", bufs=3)
                    nc.sync.dma_start(xwxt_sb[:], xwx_dram[t])
                    hT_sb = wp.tile([128, 8, 128], BF, name=f"hT{t}", tag="hT_sb",
                                    bufs=3)
                    sc_sb = wp.tile([128, P, 8], F32, name=f"sc{t}", tag="sc_sb")
                    for c2 in range(8):
                        nc.sync.dma_start(
                            hT_sb[:, c2, :],
                            gat[c2, 0:HT_BYTES].bitcast(BF)
                            .rearrange("(p b) -> p b", p=128))
                        nc.sync.dma_start(
                            sc_sb[:, :, c2],
                            gat[c2, HT_BYTES:PAY].bitcast(F32)
                            .rearrange("(n q) -> n q", n=128))
                    # softmax weights (no max subtraction: |scores| < ~10)
                    ssum = wp.tile([128, P], F32, name=f"ssum{t}", tag="ssum")
                    nc.vector.tensor_reduce(ssum[:], sc_sb[:],
                                            mybir.AxisListType.X, AluOp.add)
                    e_w = wp.tile([128, P], F32, name=f"ew{t}", tag="e_w")
                    zs = wp.tile([128, 1], F32, name=f"zs{t}", tag="zs")
                    nc.scalar.activation(e_w[:], ssum[:], Act.Exp, accum_out=zs[:])
                    rz = wp.tile([128, 1], F32, name=f"rz{t}", tag="rz")
                    nc.vector.reciprocal(rz[:], zs[:])
                    w_sb = wp.tile([128, P], F32, name=f"w{t}", tag="w_sb")
                    nc.vector.tensor_scalar_mul(w_sb[:], e_w[:], rz[:])
                    if t == 0 and DBG:
                        nc.sync.dma_start(dbg_sc[:], ssum[:])
                        nc.sync.dma_start(dbg_w[:], w_sb[:])
                    # h @ WhS
                    ps_h = psB.tile([128, SC], F32, name=f"ps_h{t}", tag="ps_mm",
                                    bufs=2)
                    for kt in range(8):
                        nc.tensor.matmul(ps_h[:], hT_sb[:, kt, :], whs_sb[:, kt, :],
                                         start=(kt == 0), stop=(kt == 7))
                    # attention FMA chain:  acc = xwxb + sum_p w_p * B_p
                    accs = [wp.tile([128, SC], BF, name=f"acc{t}_{i}", tag="accs",
                                    bufs=2) for i in range(2)]
                    nc.vector.scalar_tensor_tensor(
                        accs[0][:], b_sb[:, 0, :], w_sb[:, 0:1], xwxt_sb[:],
                        op0=AluOp.mult, op1=AluOp.add)
                    for p in range(1, P):
                        nc.vector.scalar_tensor_tensor(
                            accs[p % 2][:], b_sb[:, p, :], w_sb[:, p:p + 1],
                            accs[(p - 1) % 2][:], op0=AluOp.mult, op1=AluOp.add)
                    a_sb = wp.tile([128, SC], BF, name=f"a{t}", tag="a_sb")
                    nc.vector.tensor_add(a_sb[:], accs[(P - 1) % 2][:], ps_h[:])
                    # gates: [i | f | o | g] chunks of 128
                    if t == 0 and DBG:
                        dbg_a_s = wp.tile([128, SC], F32, name="dbg_a_s",
                                          bufs=1)
                        nc.vector.tensor_copy(dbg_a_s[:], a_sb[:])
                        nc.sync.dma_start(dbg_a[:], dbg_a_s[:])
                    si = wp.tile([128, 128], BF, name=f"si{t}", tag="si")
                    sf = wp.tile([128, 128], BF, name=f"sf{t}", tag="sf")
                    so = wp.tile([128, 128], BF, name=f"so{t}", tag="so")
                    tg = wp.tile([128, 128], BF, name=f"tg{t}", tag="tg")
                    nc.scalar.activation(si[:], a_sb[:, 0:128], Act.Sigmoid)
                    nc.scalar.activation(sf[:], a_sb[:, 128:256], Act.Sigmoid)
                    nc.scalar.activation(so[:], a_sb[:, 256:384], Act.Sigmoid)
                    nc.scalar.activation(tg[:], a_sb[:, 384:512], Act.Tanh)
                    fc = wp.tile([128, 128], F32, name=f"fc{t}", tag="fc")
                    ig = wp.tile([128, 128], F32, name=f"ig{t}", tag="ig")
                    nc.vector.tensor_mul(fc[:], sf[:], c_sb[:])
                    nc.vector.tensor_mul(ig[:], si[:], tg[:])
                    nc.vector.tensor_add(c_sb[:], fc[:], ig[:])
                    tc_t = wp.tile([128, 128], BF, name=f"tc{t}", tag="tc_t")
                    nc.scalar.activation(tc_t[:], c_sb[:], Act.Tanh)
                    h_sl = wp.tile([128, 128], BF, name=f"hsl{t}", tag="h_sl")
                    nc.vector.tensor_mul(h_sl[:], so[:], tc_t[:])
                    # transpose h slice
                    if t == 0 and DBG:
                        dbg_h1_s = wp.tile([128, 128], F32, name="dbg_h1_s",
                                           bufs=1)
                        nc.vector.tensor_copy(dbg_h1_s[:], h_sl[:])
                        nc.sync.dma_start(dbg_h1[:], dbg_h1_s[:])
                    ps_tr = psB.tile([128, 128], BF, name=f"ps_tr{t}",
                                     tag="ps_tr", bufs=1)
                    nc.tensor.transpose(ps_tr[:], h_sl[:], ident_sb[:])
                    hT_c = wp.tile([128, 128], BF, name=f"hTc{t}", tag="hT_c")
                    nc.scalar.copy(hT_c[:], ps_tr[:])
                    # scores partial for h_{t+1} + AllGather
                    gat_next = scores_and_ag(t + 1, hT_c)
                    # vocab row t uses hs[t] = h_{t+1}... but row t-1 = h_t is
                    # what we have gathered now (hT_sb).
                    if t > 0 and STAGE >= 4:
                        vocab_row(t - 1, hT_sb)
                    gat = gat_next

                # tail: vocab for last row (h_T from final AG)
                hT_last = wp.tile([128, 8, 128], BF, name="hT_last", tag="hT_sb")
                for c2 in range(8):
                    nc.sync.dma_start(
                        hT_last[:, c2, :],
                        gat[c2, 0:HT_BYTES].bitcast(BF)
                        .rearrange("(p b) -> p b", p=128))
                if STAGE >= 4:
                    vocab_row(T - 1, hT_last)

                # ---------- final loss ----------
                if os.environ.get("KFIN", "1") == "0":
                    loss_dbg2 = wp.tile([1, 1], F32, name="loss_dbg2")
                    nc.scalar.mul(loss_dbg2[:], h0t_f[0:1, 0:1], 1.0)
                    nc.sync.dma_start(loss_out[:], loss_dbg2[:])
                else:
                    zfin = wp.tile([128, T], F32, name="zfin")
                    nc.vector.tensor_reduce(zfin[:], zc_sb[:], mybir.AxisListType.X,
                                            AluOp.add)
                    if DBG:
                        nc.sync.dma_start(dbg_z[:], zfin[:])
                    nc.sync.dma_start(
                        pay2[0:ZB].bitcast(F32).rearrange("(p b) -> p b", p=128),
                        zfin[:])
                    gat2 = dram.tile([NCORES, N * T * 4 + T * 128 * 4], U8, name="gat2")
                    nc.gpsimd.collective_compute(
                        "AllGather", AluOp.bypass, replica_groups=rg,
                        ins=[pay2.opt()], outs=[gat2.opt()])
                    zg = wp.tile([128, T, 8], F32, name="zg")
                    lg = wp.tile([128, T, 8], F32, name="lg")
                    for c2 in range(8):
                        nc.sync.dma_start(
                            zg[:, :, c2],
                            gat2[c2, 0:ZB].bitcast(F32).rearrange("(n q) -> n q", n=128))
                        nc.sync.dma_start(
                            lg[:, :, c2],
                            gat2[c2, ZB:].bitcast(F32)
                            .rearrange("(n q) -> n q", n=128))
                    zred = wp.tile([128, T], F32, name="zred")
                    llred = wp.tile([128, T], F32, name="llred")
                    nc.vector.tensor_reduce(zred[:], zg[:], mybir.AxisListType.X,
                                            AluOp.add)
                    nc.vector.tensor_reduce(llred[:], lg[:], mybir.AxisListType.X,
                                            AluOp.add)
                    lse = wp.tile([128, T], F32, name="lse")
                    nc.scalar.activation(lse[:], zred[:], Act.Ln)
                    diff = wp.tile([128, T], F32, name="diff")
                    nc.vector.tensor_sub(diff[:], lse[:], llred[:])
                    nc.vector.tensor_mul(diff[:], diff[:], mask_sb[:])
                    per_n = wp.tile([128, 1], F32, name="per_n")
                    nc.vector.tensor_reduce(per_n[:], diff[:], mybir.AxisListType.X,
                                            AluOp.add)
                    pn_red = wp.tile([128, 1], F32, name="pn_red")
                    nc.gpsimd.partition_all_reduce(pn_red[:], per_n[:], 128,
                                                   bass_isa.ReduceOp.add)
                    loss_sb = wp.tile([1, 1], F32, name="loss_sb")
                    nc.scalar.mul(loss_sb[:], pn_red[0:1, :], 1.0 / N)
                    nc.sync.dma_start(loss_out[:], loss_sb[:])
            else:
                loss_dbg = wp.tile([1, 1], F32, name="loss_dbg")
                nc.scalar.mul(loss_dbg[:], h0t_f[0:1, 0:1], 1.0)
                nc.sync.dma_start(loss_out[:], loss_dbg[:])

    nc.compile()
    return nc


def host_prep(inputs, T):
    """Build the 8 per-core input maps (all numpy)."""
    g = {k: np.asarray(v) for k, v in inputs.items()}
    images, captions = g["images"], g["captions"]
    W_embed, W_proj, b_proj = g["W_embed"], g["W_proj"], g["b_proj"]
    Wx, Wh, Wattn, b = g["Wx"], g["Wh"], g["Wattn"], g["b"]
    W_vocab, b_vocab = g["W_vocab"], g["b_vocab"]

    cap = np.asarray(captions)
    cap_in = cap[:, :T]
    cap_out = cap[:, 1:T + 1]
    x_emb = W_embed[cap_in]                      # [N, T, WD]
    xembT = np.ascontiguousarray(
        x_emb.transpose(2, 1, 0).reshape(WD, T * N)).astype(BF16)
    imgsT = np.ascontiguousarray(
        images.reshape(N, CIN, P).transpose(1, 0, 2).reshape(CIN, N * P)
    ).astype(BF16)
    mask = (cap_out != 0).astype(np.float32)     # [N, T]
    ident = np.eye(128, dtype=BF16)
    ones32 = np.full((128, 32), 1.0 / 32.0, dtype=BF16)
    ones1 = np.ones((128, 1), dtype=BF16)
    ones1f = np.ones((128, 1), dtype=np.float32)
    onesrow = np.ones((1, 128), dtype=BF16)
    identf = np.eye(128, dtype=np.float32)

    # label weight vectors, [H, N, T] -> per t: [hl, (n, kt)]
    wy = W_vocab[:, cap_out]                     # [H, N, T]
    wy_t = wy.reshape(8, 128, N, T).transpose(3, 1, 2, 0)  # [T, hl, n, kt]

    in_maps = []
    for c in range(NCORES):
        hsl = slice(128 * c, 128 * (c + 1))
        idx = np.concatenate([g4 * H + 128 * c + np.arange(128)
                              for g4 in range(4)])
        vsl = slice(VS * c, VS * (c + 1))
        wyc = wy_t.copy()
        nm = np.zeros(N, dtype=wy_t.dtype)
        nm[16 * c:16 * (c + 1)] = 1
        wyc *= nm[None, None, :, None]
        in_maps.append({
            "imgsT": imgsT,
            "wprojS": np.ascontiguousarray(W_proj[:, hsl]).astype(BF16),
            "bprojS": np.ascontiguousarray(b_proj[hsl, None]).astype(np.float32),
            "xembT": xembT,
            "WxS": np.ascontiguousarray(Wx[:, idx]).astype(BF16),
            "bSb": np.ascontiguousarray(
                np.broadcast_to(b[idx], (N, SC))).astype(np.float32),
            "WhS": np.ascontiguousarray(Wh[:, idx]).astype(BF16),
            "WattnS": np.ascontiguousarray(Wattn[:, idx]).astype(BF16),
            "wvoc": np.ascontiguousarray(W_vocab[:, vsl]).astype(BF16),
            "bvoc": np.ascontiguousarray(b_vocab[None, vsl]).astype(BF16),
            "wyT": np.ascontiguousarray(
                wyc.reshape(T, 128, H)).astype(BF16),
            "mask_f": mask,
            "ident": ident,
            "ones32": ones32,
            "ones1": ones1,
            "ones1f": ones1f,
            "onesrow": onesrow,
            "identf": identf,
        })
    host_by = float(np.sum(mask.astype(np.float64) *
                           np.asarray(b_vocab, np.float64)[cap_out]) / N)
    return in_maps, host_by


_CACHE = {}


def _get_built(T):
    if T not in _CACHE:
        _CACHE[T] = build(T)
    return _CACHE[T]


def run(inputs, T=30):
    nc = _get_built(T)
    in_maps, host_by = host_prep(inputs, T)
    res = run_bass_kernel_spmd(nc, in_maps, core_ids=list(range(NCORES)))
    dev_loss = float(res.results[0]["loss"][0, 0])
    return np.float32(dev_loss - host_by)


def kernel(**inputs) -> np.ndarray:
    return run(inputs, T=30)

